# revision 1
# baseline (speedup 1.0000x reference)
import jax
import jax.numpy as jnp
import numpy as np
from functools import partial

# Hardcoded problem shapes (nn_GTM_15702400434566)
B, L, IN_DIM, OUT_DIM = 8, 2048, 1024, 64
H = 2
NN = 32
LN_EPS = 1e-5


def _layer_norm(x, g, b):
    m = jnp.mean(x, axis=-1, keepdims=True)
    v = jnp.mean((x - m) ** 2, axis=-1, keepdims=True)
    return (x - m) * jax.lax.rsqrt(v + LN_EPS) * g + b


def _self_attention(x, add_mask, top, ln_g, ln_b):
    # x: [L,D]; add_mask: [L]; top: [L,L] 0/1 mask of top-NN neighbors per row
    Lx, D = x.shape
    d = D // H
    h = x.reshape(Lx, H, d).transpose(1, 0, 2)                 # [H,L,d]
    scores = jnp.einsum('hid,hjd->hij', h, h) + add_mask[None, None, :]
    probs = jax.nn.softmax(scores, axis=-1)
    probs = probs * top[None, :, :]
    probs = probs / (jnp.sum(probs, axis=-1, keepdims=True) + 1e-5)
    out = jnp.einsum('hij,hjd->hid', probs, h)
    out = out.transpose(1, 0, 2).reshape(Lx, D)
    return _layer_norm(out, ln_g, ln_b)


def _fwd_one(node, dist, mask, w):
    # node: [L,IN_DIM], dist: [L,L], mask: [L]; w: dict of replicated weights
    x = _layer_norm(node, w['ln_in_g'], w['ln_in_b'])
    x = jax.nn.leaky_relu(x @ w['w_in'] + w['b_in'], 0.01)
    x = _layer_norm(x, w['ln_h1_g'], w['ln_h1_b'])
    x = jax.nn.leaky_relu(x @ w['w_h'] + w['b_h'], 0.01)
    x = _layer_norm(x, w['ln_h2_g'], w['ln_h2_b'])

    # distance weights; row-normalization does not change per-row ranks,
    # so the top-NN mask can be computed from the unnormalized weights.
    dw = (1.0 / jnp.sqrt(1.0 + dist)) * mask[None, :]          # [L,L]
    kth = jax.lax.top_k(dw, NN)[0][:, NN - 1]                  # [L]
    top = (dw >= kth[:, None]).astype(jnp.float32)             # [L,L]

    add_mask = (1.0 - mask) * -10000.0                         # [L]
    x = _self_attention(x, add_mask, top, w['ln_a0_g'], w['ln_a0_b'])
    x = _self_attention(x, add_mask, top, w['ln_a1_g'], w['ln_a1_b'])
    y = (x @ w['w_out'] + w['b_out']).squeeze(-1)              # [L]
    return y


_WKEYS = ('ln_in_g', 'ln_in_b', 'w_in', 'b_in', 'ln_h1_g', 'ln_h1_b',
          'w_h', 'b_h', 'ln_h2_g', 'ln_h2_b', 'ln_a0_g', 'ln_a0_b',
          'ln_a1_g', 'ln_a1_b', 'w_out', 'b_out')

_pmapped = None
_jitted = None


def _get_pmapped():
    global _pmapped
    if _pmapped is None:
        _pmapped = jax.pmap(_fwd_one, in_axes=(0, 0, 0, None))
    return _pmapped


def _get_jitted():
    global _jitted
    if _jitted is None:
        _jitted = jax.jit(jax.vmap(_fwd_one, in_axes=(0, 0, 0, None)))
    return _jitted


def kernel(**inputs):
    node = jnp.asarray(inputs['protein_node_features'], jnp.float32)
    dist = jnp.asarray(inputs['protein_dist_matrix'], jnp.float32)
    mask = jnp.asarray(inputs['protein_masks'], jnp.float32)
    w = {k: jnp.asarray(inputs[k], jnp.float32) for k in _WKEYS}

    # Data-parallel over batch B=8: one batch element per NeuronCore,
    # tiny 64-dim weights replicated on every core.
    if len(jax.devices()) >= B and node.shape[0] == B:
        y = _get_pmapped()(node, dist, mask, w)
    else:
        y = _get_jitted()(node, dist, mask, w)
    return np.asarray(y, np.float32)

